# revision 3
# baseline (speedup 1.0000x reference)
"""GAT encoder (10-layer, JK-concat) Trainium2 Bass kernel v2 — 8-core.

One collective per layer: AllGather PRE-pairnorm aggregated rows (transposed
form, bf16) with f32 stats packed via bitcast into extra columns. Each core
then runs pairnorm+gelu+dense redundantly over the full 50176-node table to
build its local gather table [h(128) | one | pad | zs(f32-as-2bf16)].
Attention logit zs is gathered WITH the features (no per-edge dots); the
softmax denominator rides as a ones-column through the PE accumulate matmul.
Pads point at dedicated zs=-200 rows so their exp weight underflows to 0.
"""

import numpy as np
import ml_dtypes
from contextlib import ExitStack

import concourse.bass as bass
import concourse.bacc as bacc
import concourse.tile as tile
import concourse.mybir as mybir

F32 = mybir.dt.float32
F32R = mybir.dt.float32r
BF16 = mybir.dt.bfloat16
I16 = mybir.dt.int16
AX = mybir.AxisListType
OP = mybir.AluOpType
AF = mybir.ActivationFunctionType

N = 50000
E = 640000
HID = 128
L = 10
NC = 8
NSH = N // NC          # 6250
TILES = 49
NSHP = TILES * 128     # 6272
TBL = NSHP * NC        # 50176
HI_BASE = TBL - 32768  # 17408
NEG = 0.2
PEPS = 1e-5
SEPS = 1e-16
SEG_MAX_ROUNDS = 48
ROWW = 256             # table row (bf16): h(128)|one|pad|zs f32|pad... 512B
GW = 256               # gather window = table row
PSW = 132              # psum row width: h(128) | 0 | 0 | zs | zd
OWNW = 134             # own row:   h(128) | one | pad | zs f32 | zd f32
ZS_F32 = 65            # f32 col of zs within a row (bytes 260..263)
PAD_LO_ROW = 6250          # core0 pad row (zs=-200), in lo window
PAD_HI_ROW = TBL - 22      # 50154 core7 pad row; 50154-17408=32746 <= 32767
PAD_SLOT0 = NSH - (TILES - 1) * 128   # 106: first pad slot in tile 48
STATS_COLS = 4
AGW = NSHP + STATS_COLS
# stream chunks per block (cols), multiples of 128
CHUNKS = [(0, 1664), (1664, 3328), (3328, 4992), (4992, 6272)]
import os
NO_COLL = os.environ.get("K2_NO_COLL", "0") == "1"
PAIR_MM = os.environ.get("K2_PAIR_MM", "0") == "1" 


def preprocess(edge_index):
    """Static graph preprocessing (no mask needed in v2)."""
    src = np.asarray(edge_index[0], dtype=np.int64)
    dst = np.asarray(edge_index[1], dtype=np.int64)
    owner = dst // NSH

    orders = []
    inv_all = np.empty(N, np.int64)
    for c in range(NC):
        m = owner == c
        dloc = dst[m] - c * NSH
        deg = np.bincount(dloc, minlength=NSH)
        order = np.argsort(-deg, kind="stable")
        inv = np.empty(NSH, np.int64)
        inv[order] = np.arange(NSH)
        orders.append(order)
        inv_all[c * NSH:(c + 1) * NSH] = inv
    tblrow_of_src = (src // NSH) * NSHP + inv_all[src]

    lo_lists = [[[] for _ in range(NSHP)] for _ in range(NC)]
    hi_lists = [[[] for _ in range(NSHP)] for _ in range(NC)]
    for c in range(NC):
        m = owner == c
        rows = tblrow_of_src[m]
        dpos = inv_all[dst[m]]
        o = np.argsort(dpos, kind="stable")
        rows = rows[o]
        dpos = dpos[o]
        counts = np.bincount(dpos, minlength=NSH)
        starts = np.concatenate([[0], np.cumsum(counts)])
        for p in range(NSH):
            r = rows[starts[p]:starts[p + 1]]
            ml = r[r < HI_BASE]
            mh = r[r > 32767]
            fx = r[(r >= HI_BASE) & (r <= 32767)]
            nl, nh = len(ml), len(mh)
            lo_e, hi_e = [], []
            for v in fx:
                if nl <= nh:
                    lo_e.append(v); nl += 1
                else:
                    hi_e.append(v); nh += 1
            lo_lists[c][p] = np.concatenate([ml, np.array(lo_e, np.int64)]) if (len(ml) + len(lo_e)) else np.empty(0, np.int64)
            hi_lists[c][p] = np.concatenate([mh, np.array(hi_e, np.int64)]) if (len(mh) + len(hi_e)) else np.empty(0, np.int64)

    D_lo = np.zeros(TILES, np.int64)
    D_hi = np.zeros(TILES, np.int64)
    for t in range(TILES):
        for c in range(NC):
            for sl in range(128):
                p = t * 128 + sl
                D_lo[t] = max(D_lo[t], len(lo_lists[c][p]))
                D_hi[t] = max(D_hi[t], len(hi_lists[c][p]))
    rounds_tot = int((D_lo + D_hi).sum())

    segs = []
    cur, cur_r = [], 0
    for t in range(TILES):
        rt = int(D_lo[t] + D_hi[t])
        if cur and cur_r + rt > SEG_MAX_ROUNDS:
            segs.append(cur)
            cur, cur_r = [], 0
        cur.append(t)
        cur_r += rt
    if cur:
        segs.append(cur)

    def wrap_idx(flat):
        n = len(flat)
        assert n % 16 == 0
        w = np.asarray(flat, np.int16).reshape(-1, 16).T
        return np.tile(w, (8, 1))

    percore = []
    for c in range(NC):
        idx_blocks = []
        for seg in segs:
            for part, D in (("lo", D_lo), ("hi", D_hi)):
                flat = []
                lists = lo_lists[c] if part == "lo" else hi_lists[c]
                base = 0 if part == "lo" else HI_BASE
                padv = PAD_LO_ROW if part == "lo" else PAD_HI_ROW - HI_BASE
                for t in seg:
                    for k in range(int(D[t])):
                        for sl in range(128):
                            p = t * 128 + sl
                            lst = lists[p]
                            if k < len(lst):
                                flat.append(int(lst[k]) - base)
                            else:
                                flat.append(padv)
                if flat:
                    idx_blocks.append(wrap_idx(flat))
        idx_all = np.concatenate(idx_blocks, axis=1) if idx_blocks else np.zeros((128, 1), np.int16)
        percore.append({"idx": idx_all, "order": orders[c]})

    meta = {"D_lo": D_lo, "D_hi": D_hi, "segs": segs, "rounds_tot": rounds_tot}
    return meta, percore


def build(nc, meta, n_layers=L):
    D_lo, D_hi, segs = meta["D_lo"], meta["D_hi"], meta["segs"]
    rounds_tot = meta["rounds_tot"]

    # ---- DRAM tensors
    xag0_in = nc.dram_tensor("xag0", [NC * 128, NSHP], BF16, kind="ExternalInput")
    ownx0_in = nc.dram_tensor("ownx0", [128, NSHP], BF16, kind="ExternalInput")
    idx_in = nc.dram_tensor("idx", [128, 8 * rounds_tot], I16, kind="ExternalInput")
    Wb_in = nc.dram_tensor("Wb", [n_layers, 128, 128], BF16, kind="ExternalInput")
    WTb_in = nc.dram_tensor("WTb", [n_layers, 128, 128], BF16, kind="ExternalInput")
    avec_in = nc.dram_tensor("avec", [n_layers, 128, 2], BF16, kind="ExternalInput")
    biasr_in = nc.dram_tensor("biasr", [n_layers, 128, 128], F32, kind="ExternalInput")
    lwb_in = nc.dram_tensor("lwb", [n_layers, 128, 128], BF16, kind="ExternalInput")
    linb_in = nc.dram_tensor("linb", [128, 1], F32, kind="ExternalInput")
    identb_in = nc.dram_tensor("identb", [128, 128], BF16, kind="ExternalInput")
    identf_in = nc.dram_tensor("identf", [128, 128], F32, kind="ExternalInput")
    onesb_in = nc.dram_tensor("onesb", [128, 128], F32, kind="ExternalInput")
    padm_in = nc.dram_tensor("padm", [128, 1], F32, kind="ExternalInput")
    padz_in = nc.dram_tensor("padz", [128, 1], F32, kind="ExternalInput")
    y_out = nc.dram_tensor("y", [NSHP, 128], F32, kind="ExternalOutput")

    ag_in = nc.dram_tensor("ag_in", [128, AGW], BF16)
    xag = nc.dram_tensor("xag", [NC * 128, AGW], BF16, addr_space="Shared")
    ag_st = nc.dram_tensor("ag_st", [128, STATS_COLS], BF16)
    xag_st = nc.dram_tensor("xag_st", [NC * 128, STATS_COLS], BF16,
                            addr_space="Shared")
    table = nc.dram_tensor("table", [TBL, ROWW], BF16)

    RG = [list(range(NC))]

    with tile.TileContext(nc) as tc, ExitStack() as ctx:
        P = ctx.enter_context(tc.tile_pool(name="persist", bufs=1))
        idx_sb = P.tile([128, 8 * rounds_tot], I16, tag="idx")
        Wb = P.tile([128, n_layers * 128], BF16, tag="Wb")
        WTb = P.tile([128, n_layers * 128], BF16, tag="WTb")
        avec = P.tile([128, n_layers * 2], BF16, tag="avec")
        biasr = P.tile([128, n_layers * 128], F32, tag="biasr")
        lwb = P.tile([128, n_layers * 128], BF16, tag="lwb")
        linb = P.tile([128, 1], F32, tag="linb")
        identb = P.tile([128, 128], BF16, tag="identb")
        identf = P.tile([128, 128], F32, tag="identf")
        onesb = P.tile([128, 128], F32, tag="onesb")
        padm = P.tile([128, 1], F32, tag="padm")
        padz = P.tile([128, 1], F32, tag="padz")
        ownxT = P.tile([128, NSHP], BF16, tag="ownxT")
        ownh = P.tile([128, TILES * OWNW], BF16, tag="ownh")
        payload = P.tile([128, AGW], BF16, tag="payload")
        outfinT = P.tile([128, NSHP], F32, tag="outfinT")
        wself = P.tile([128, TILES], F32, tag="wself")
        sqparts = P.tile([128, 13], F32, tag="sqparts")
        invdr = P.tile([128, 1], F32, tag="invdr")
        nms = P.tile([128, 1], F32, tag="nms")
        Waug = P.tile([128, PSW], BF16, tag="Waug")
        scr = P.tile([128, 512], F32, tag="scr")

        nc.sync.dma_start(idx_sb[:], idx_in.ap())
        nc.sync.dma_start(Wb[:].rearrange("a (l b) -> a l b", b=128), Wb_in.ap().rearrange("l a b -> a l b"))
        nc.sync.dma_start(WTb[:].rearrange("a (l b) -> a l b", b=128), WTb_in.ap().rearrange("l a b -> a l b"))
        nc.sync.dma_start(avec[:].rearrange("a (l b) -> a l b", b=2), avec_in.ap().rearrange("l a b -> a l b"))
        nc.sync.dma_start(biasr[:].rearrange("a (l b) -> a l b", b=128), biasr_in.ap().rearrange("l a b -> a l b"))
        nc.sync.dma_start(lwb[:].rearrange("a (l b) -> a l b", b=128), lwb_in.ap().rearrange("l a b -> a l b"))
        nc.sync.dma_start(linb[:], linb_in.ap())
        nc.sync.dma_start(identb[:], identb_in.ap())
        nc.sync.dma_start(identf[:], identf_in.ap())
        nc.sync.dma_start(onesb[:], onesb_in.ap())
        nc.sync.dma_start(padm[:], padm_in.ap())
        nc.sync.dma_start(padz[:], padz_in.ap())
        nc.sync.dma_start(ownxT[:], ownx0_in.ap())

        PR = ctx.enter_context(tc.tile_pool(name="pr", bufs=2, space="PSUM"))
        PA = ctx.enter_context(tc.tile_pool(name="pa", bufs=2, space="PSUM"))
        PT = ctx.enter_context(tc.tile_pool(name="pt", bufs=2, space="PSUM"))
        GSEG = ctx.enter_context(tc.tile_pool(name="gseg", bufs=2))
        XIN = ctx.enter_context(tc.tile_pool(name="xin", bufs=2))
        RS = ctx.enter_context(tc.tile_pool(name="rs", bufs=2))
        SC = ctx.enter_context(tc.tile_pool(name="sc", bufs=4))
        ST = ctx.enter_context(tc.tile_pool(name="st", bufs=6))
        RP = ctx.enter_context(tc.tile_pool(name="rp", bufs=4))

        # idx col offsets per segment (wrapped layout: 8 cols per round)
        idx_off = {}
        ic = 0
        for si, seg in enumerate(segs):
            lo_r = int(sum(D_lo[t] for t in seg))
            hi_r = int(sum(D_hi[t] for t in seg))
            idx_off[si] = (ic, lo_r, ic + 8 * lo_r, hi_r)
            ic += 8 * (lo_r + hi_r)

        def stats_prologue(from_st=False):
            gst = ST.tile([128, 4 * NC], BF16, tag="gst")
            src_ap = (xag_st.ap() if from_st
                      else xag.ap()[:, NSHP:NSHP + 4])
            nc.sync.dma_start(
                gst[:].rearrange("p (c f) -> p c f", f=4),
                src_ap.rearrange("(c p) f -> p c f", p=128))
            gf = gst[:].bitcast(F32).rearrange("p (c f) -> p c f", f=2)
            Sx = ST.tile([128, 1], F32, tag="Sx")
            nc.vector.reduce_sum(out=Sx[:], in_=gf[:, :, 0:1].squeeze(2), axis=AX.X)
            Sq = ST.tile([128, 1], F32, tag="Sq")
            nc.vector.reduce_sum(out=Sq[:], in_=gf[:, :, 1:2].squeeze(2), axis=AX.X)
            mu = ST.tile([128, 1], F32, tag="mu")
            nc.vector.tensor_scalar_mul(mu[:], Sx[:], 1.0 / N)
            p1 = PT.tile([128, 128], F32, tag="pt")
            nc.tensor.matmul(p1[:1, :1], Sq[:], onesb[:, :1], start=True, stop=True)
            p2 = PT.tile([128, 128], F32, tag="pt")
            nc.tensor.matmul(p2[:1, :1], mu[:], mu[:], start=True, stop=True)
            sqt = ST.tile([1, 2], F32, tag="sqt")
            nc.vector.tensor_copy(sqt[:, 0:1], p1[:1, :1])
            nc.vector.tensor_copy(sqt[:, 1:2], p2[:1, :1])
            v3 = ST.tile([1, 1], F32, tag="v3")
            nc.vector.tensor_scalar(v3[:], sqt[:, 0:1], 1.0 / N, PEPS, op0=OP.mult, op1=OP.add)
            v4 = ST.tile([1, 1], F32, tag="v4")
            nc.vector.tensor_tensor(v4[:], v3[:], sqt[:, 1:2], op=OP.subtract)
            den = ST.tile([1, 1], F32, tag="den")
            nc.scalar.activation(den[:], v4[:], AF.Sqrt)
            invd = ST.tile([1, 1], F32, tag="invd")
            nc.vector.reciprocal(invd[:], den[:])
            pb = PT.tile([128, 128], F32, tag="pt")
            nc.tensor.matmul(pb[:, :1], onesb[:1, :], invd[:], start=True, stop=True)
            nc.vector.tensor_copy(invdr[:], pb[:, :1])
            nc.vector.tensor_scalar(nms[:], mu[:], invdr[:], -1.0, op0=OP.mult, op1=OP.mult)

        def build_waug(l):
            pw = PT.tile([128, 128], F32, tag="pt")
            nc.tensor.matmul(pw[:, :2], WTb[:, l * 128:(l + 1) * 128],
                             avec[:, l * 2:(l + 1) * 2], start=True, stop=True)
            nc.vector.tensor_copy(Waug[:, :128], Wb[:, l * 128:(l + 1) * 128])
            nc.vector.memset(Waug[:, 128:130], 0)
            nc.vector.tensor_copy(Waug[:, 130:132], pw[:, :2])

        def stream_block(l, c):
            """Build table rows for block c of the full node table."""
            src_hbm = xag0_in if l == 0 else xag
            for (c0, c1) in CHUNKS:
                ncols = c1 - c0
                ntl = ncols // 128
                xin = XIN.tile([128, 1664], BF16, tag="xin")
                nc.sync.dma_start(xin[:, :ncols],
                                  src_hbm.ap()[c * 128:(c + 1) * 128, c0:c1])
                if l == 0:
                    xs = xin
                else:
                    xs = XIN.tile([128, 1664], BF16, tag="xs")
                    nc.scalar.activation(xs[:, :ncols], xin[:, :ncols],
                                         AF.Gelu, bias=nms[:], scale=invdr[:])
                rows = RS.tile([128, 13 * ROWW], BF16, tag="rows")
                for g0 in range(0, ntl, 3):
                    gn = min(3, ntl - g0)
                    pr = PR.tile([128, 3 * PSW], F32, tag="pr")
                    for j in range(gn):
                        nc.tensor.matmul(
                            pr[:, j * PSW:j * PSW + PSW],
                            xs[:, (g0 + j) * 128:(g0 + j + 1) * 128],
                            Waug[:], start=True, stop=True)
                    pr3 = pr[:].rearrange("p (k f) -> p k f", f=PSW)
                    r3 = rows[:, g0 * ROWW:].rearrange("p (k f) -> p k f", f=ROWW)
                    if g0 % 2 == 0:
                        nc.vector.tensor_copy(r3[:, :gn, :128], pr3[:, :gn, :128])
                    else:
                        nc.scalar.activation(r3[:, :gn, :128], pr3[:, :gn, :128], AF.Copy)
                    nc.vector.memset(r3[:, :gn, 128:130], 1.0)
                    rzf = rows[:].bitcast(F32).rearrange("p (k f) -> p k f", f=ROWW // 2)
                    nc.vector.tensor_copy(rzf[:, g0:g0 + gn, 65:66],
                                          pr3[:, :gn, 130:131])
                # pad-row zs fix on global tile 48 of this block (last chunk)
                if c1 == NSHP:
                    jt = ntl - 1
                    zv = rows[:].bitcast(F32).rearrange(
                        "p (k f) -> p k f", f=ROWW // 2)[:, jt:jt + 1, 65:66].squeeze(2)
                    nc.vector.tensor_scalar(zv, zv, padm[:], padz[:],
                                            op0=OP.mult, op1=OP.add)
                nc.sync.dma_start(
                    table.ap()[c * NSHP + c0:c * NSHP + c1, :].rearrange(
                        "(k p) f -> p k f", p=128),
                    rows[:, :ntl * ROWW].rearrange("p (k f) -> p k f", f=ROWW))

        def own_dense(l):
            """ownh slab: h(128)|..|zs,zd(f32) per own tile from ownxT."""
            for g0 in range(0, TILES, 3):
                gn = min(3, TILES - g0)
                pr = PR.tile([128, 3 * PSW], F32, tag="pr")
                for j in range(gn):
                    nc.tensor.matmul(
                        pr[:, j * PSW:j * PSW + PSW],
                        ownxT[:, (g0 + j) * 128:(g0 + j + 1) * 128],
                        Waug[:], start=True, stop=True)
                pr3 = pr[:].rearrange("p (k f) -> p k f", f=PSW)
                o3 = ownh[:, g0 * OWNW:].rearrange("p (k f) -> p k f", f=OWNW)
                nc.vector.tensor_copy(o3[:, :gn, :128], pr3[:, :gn, :128])
                zof = ownh[:].bitcast(F32).rearrange("p (k f) -> p k f", f=OWNW // 2)
                nc.vector.tensor_copy(zof[:, g0:g0 + gn, 65:67],
                                      pr3[:, :gn, 130:132])

        def own_norm(l):
            # ownxT <- gelu(pairnorm(payload prenorm))  [l>=1]
            for c0 in range(0, NSHP, 512):
                cn = min(512, NSHP - c0)
                nc.scalar.activation(ownxT[:, c0:c0 + cn], payload[:, c0:c0 + cn],
                                     AF.Gelu, bias=nms[:], scale=invdr[:])

        def jk_increment(jl):
            for c0 in range(0, NSHP, 512):
                cn = min(512, NSHP - c0)
                pj = PA.tile([128, 512], F32, tag="pj")
                nc.tensor.matmul(pj[:, :cn], lwb[:, jl * 128:(jl + 1) * 128],
                                 ownxT[:, c0:c0 + cn], start=True, stop=True)
                if jl == 0:
                    nc.vector.tensor_copy(outfinT[:, c0:c0 + cn], pj[:, :cn])
                else:
                    nc.vector.tensor_tensor(outfinT[:, c0:c0 + cn],
                                            outfinT[:, c0:c0 + cn], pj[:, :cn], op=OP.add)

        def wself_compute():
            ozf = ownh[:].bitcast(F32).rearrange("p (k f) -> p k f", f=OWNW // 2)
            zsum = ST.tile([128, TILES], F32, tag="zsum")
            nc.vector.tensor_tensor(zsum[:], ozf[:, :, 65:66].squeeze(2),
                                    ozf[:, :, 66:67].squeeze(2), op=OP.add)
            zl = ST.tile([128, TILES], F32, tag="zlw")
            nc.vector.scalar_tensor_tensor(out=zl[:], in0=zsum[:], scalar=NEG,
                                           in1=zsum[:], op0=OP.mult, op1=OP.max)
            nc.scalar.activation(wself[:], zl[:], AF.Exp)

        def aggregate(l, last=False):
            for si, seg in enumerate(segs):
                lo_r = int(sum(D_lo[t] for t in seg))
                hi_r = int(sum(D_hi[t] for t in seg))
                seg_r = lo_r + hi_r
                gbuf = GSEG.tile([128, SEG_MAX_ROUNDS * GW], BF16, tag="gseg")
                g3 = gbuf[:].rearrange("p (r e) -> p r e", e=GW)
                ic_lo, nlo, ic_hi, nhi = idx_off[si]
                if lo_r:
                    nc.gpsimd.dma_gather(
                        g3[:, :lo_r, :], table.ap()[:32768, :],
                        idx_sb[:, ic_lo:ic_lo + 8 * lo_r],
                        128 * lo_r, 128 * lo_r, GW, single_packet=False)
                if hi_r:
                    nc.gpsimd.dma_gather(
                        g3[:, lo_r:seg_r, :], table.ap()[HI_BASE:TBL, :],
                        idx_sb[:, ic_hi:ic_hi + 8 * hi_r],
                        128 * hi_r, 128 * hi_r, GW, single_packet=False)

                ozf = ownh[:].bitcast(F32).rearrange("p (k f) -> p k f", f=OWNW // 2)
                zdexp = SC.tile([128, SEG_MAX_ROUNDS], F32, tag="zdexp")
                lo_cum, hi_cum = 0, 0
                tile_ranges = []
                for t in seg:
                    dlo, dhi = int(D_lo[t]), int(D_hi[t])
                    zd = ozf[:, t:t + 1, 66:67].squeeze(2)
                    if dlo:
                        nc.vector.tensor_copy(
                            zdexp[:, lo_cum:lo_cum + dlo],
                            zd.broadcast_to([128, dlo]))
                    if dhi:
                        nc.vector.tensor_copy(
                            zdexp[:, lo_r + hi_cum:lo_r + hi_cum + dhi],
                            zd.broadcast_to([128, dhi]))
                    tile_ranges.append((t, lo_cum, dlo, lo_r + hi_cum, dhi))
                    lo_cum += dlo
                    hi_cum += dhi

                zraw = gbuf[:].bitcast(F32).rearrange(
                    "p (r e) -> p r e", e=GW // 2)[:, :seg_r, 65:66].squeeze(2)
                zt = SC.tile([128, SEG_MAX_ROUNDS], F32, tag="zt")
                nc.vector.tensor_tensor(zt[:, :seg_r], zraw, zdexp[:, :seg_r], op=OP.add)
                zl = SC.tile([128, SEG_MAX_ROUNDS], F32, tag="zl")
                nc.vector.scalar_tensor_tensor(out=zl[:, :seg_r], in0=zt[:, :seg_r],
                                               scalar=NEG, in1=zt[:, :seg_r],
                                               op0=OP.mult, op1=OP.max)
                ew = SC.tile([128, SEG_MAX_ROUNDS], F32, tag="ew")
                nc.scalar.activation(ew[:, :seg_r], zl[:, :seg_r], AF.Exp)
                # apply weights in place on h+one cols
                gho = g3[:, :seg_r, 0:129]
                nc.vector.tensor_tensor(
                    gho, gho,
                    ew[:, :seg_r].unsqueeze(2).broadcast_to([128, seg_r, 129]),
                    op=OP.mult)

                for (t, lc, dlo, hc, dhi) in tile_ranges:
                    Dt = dlo + dhi
                    if PAIR_MM and dlo >= 2 and dhi >= 2:
                        pa = PA.tile([128, 258], F32, tag="pa2")
                        pa3 = pa[:].rearrange("p (a b) -> p a b", b=129)
                        mms = []
                        for (base, d) in ((lc, dlo), (hc, dhi)):
                            for k in range(0, d - 1, 2):
                                mms.append((g3[:, base + k:base + k + 2, 0:129], True))
                            if d % 2:
                                mms.append((gbuf[:, (base + d - 1) * GW:
                                                 (base + d - 1) * GW + 129], False))
                        for i, (mv, dual) in enumerate(mms):
                            nc.tensor.matmul(pa3[:, :, :] if dual else pa[:, :129],
                                             identb[:], mv,
                                             start=(i == 0), stop=(i == len(mms) - 1),
                                             skip_group_check=True)
                        mg = RP.tile([128, 129], F32, tag="mg")
                        nc.vector.tensor_tensor(mg[:], pa[:, :129], pa[:, 129:258],
                                                op=OP.add)
                        pa_h, pa_s = mg[:, :128], mg[:, 128:129]
                    else:
                        pa = PA.tile([128, 129], F32, tag="pa")
                        ks = [lc + k for k in range(dlo)] + [hc + k for k in range(dhi)]
                        for i, kc in enumerate(ks):
                            nc.tensor.matmul(pa[:], identb[:],
                                             gbuf[:, kc * GW:kc * GW + 129],
                                             start=(i == 0), stop=(i == Dt - 1))
                        pa_h, pa_s = pa[:, :128], pa[:, 128:129]
                    Sp = ST.tile([128, 1], F32, tag="Sp")
                    nc.vector.tensor_scalar(Sp[:], pa_s, wself[:, t:t + 1],
                                            SEPS, op0=OP.add, op1=OP.add)
                    rec = ST.tile([128, 1], F32, tag="rec")
                    nc.vector.reciprocal(rec[:], Sp[:])
                    acc1 = RP.tile([128, 128], F32, tag="acc1")
                    nc.vector.scalar_tensor_tensor(
                        out=acc1[:], in0=ownh[:, t * OWNW:t * OWNW + 128],
                        scalar=wself[:, t:t + 1], in1=pa_h,
                        op0=OP.mult, op1=OP.add)
                    row = RP.tile([128, 128], F32, tag="row")
                    nc.vector.scalar_tensor_tensor(
                        out=row[:], in0=acc1[:], scalar=rec[:],
                        in1=biasr[:, l * 128:(l + 1) * 128], op0=OP.mult, op1=OP.add)
                    if t == TILES - 1:
                        nc.vector.tensor_scalar_mul(row[:], row[:], padm[:])
                    ptf = PT.tile([128, 128], F32, tag="pt")
                    nc.tensor.transpose(ptf[:], row[:], identf[:])
                    nc.vector.tensor_copy(payload[:, t * 128:(t + 1) * 128], ptf[:])
            # stats from payload (per-feature sums; per-partition sumsq partials)
            pzf = payload[:].bitcast(F32)
            fsum = ST.tile([128, 1], F32, tag="fsum")
            nc.vector.reduce_sum(out=fsum[:], in_=payload[:, :NSHP], axis=AX.X)
            nc.vector.tensor_copy(pzf[:, NSHP // 2:NSHP // 2 + 1], fsum[:])
            for i, c0 in enumerate(range(0, NSHP, 512)):
                cn = min(512, NSHP - c0)
                nc.scalar.activation(scr[:, :cn], payload[:, c0:c0 + cn],
                                     AF.Square, accum_out=sqparts[:, i:i + 1])
            spt = ST.tile([128, 1], F32, tag="spt")
            nc.vector.reduce_sum(out=spt[:], in_=sqparts[:], axis=AX.X)
            nc.vector.tensor_copy(pzf[:, NSHP // 2 + 1:NSHP // 2 + 2], spt[:])
            if last:
                nc.sync.dma_start(ag_st.ap(), payload[:, NSHP:NSHP + STATS_COLS])
                nc.gpsimd.collective_compute(
                    "AllGather", OP.bypass, replica_groups=RG,
                    ins=[ag_st.ap()], outs=[xag_st.ap()])
                return
            nc.sync.dma_start(ag_in.ap(), payload[:])
            if NO_COLL:
                for c in range(NC):
                    nc.sync.dma_start(xag.ap()[c * 128:(c + 1) * 128, :], ag_in.ap())
            else:
                nc.gpsimd.collective_compute(
                    "AllGather", OP.bypass, replica_groups=RG,
                    ins=[ag_in.ap()], outs=[xag.ap()])

        # ================= main layer loop =================
        for l in range(n_layers):
            if l > 0:
                stats_prologue()
                own_norm(l)
            build_waug(l)
            # own dense first (aggregation needs ownh/wself)
            own_dense(l)
            wself_compute()
            if l > 0:
                jk_increment(l - 1)
            for c in range(NC):
                stream_block(l, c)
            aggregate(l, last=(l == n_layers - 1))

        # ================= epilogue pass =================
        stats_prologue(from_st=True)
        own_norm(n_layers)
        jk_increment(n_layers - 1)
        nc.vector.tensor_scalar_add(outfinT[:], outfinT[:], linb[:])
        for t0 in range(0, TILES, 13):
            tn = min(13, TILES - t0)
            yrows = RS.tile([128, 13 * 128], F32, tag="yrows")
            for j in range(tn):
                pt = PT.tile([128, 128], F32, tag="pt")
                nc.tensor.transpose(pt[:], outfinT[:, (t0 + j) * 128:(t0 + j + 1) * 128], identf[:])
                nc.vector.tensor_copy(yrows[:, j * 128:(j + 1) * 128], pt[:])
            nc.sync.dma_start(
                y_out.ap()[t0 * 128:(t0 + tn) * 128, :].rearrange("(t p) f -> p t f", p=128),
                yrows[:, :tn * 128].rearrange("p (t f) -> p t f", f=128))

    return nc


def make_inputs(inputs, meta, percore, n_layers=L):
    x = np.asarray(inputs["x"], np.float32)
    W0 = np.asarray(inputs["W0"], np.float32)
    Ws = np.asarray(inputs["Ws"], np.float32)
    att_src = np.asarray(inputs["att_src"], np.float32)
    att_dst = np.asarray(inputs["att_dst"], np.float32)
    bias = np.asarray(inputs["bias"], np.float32)
    lin_w = np.asarray(inputs["lin_w"], np.float32)
    lin_b = np.asarray(inputs["lin_b"], np.float32)

    Wst = np.stack([W0] + [Ws[i] for i in range(n_layers - 1)])
    Wb = Wst.astype(ml_dtypes.bfloat16)
    WTb = np.stack([Wst[i].T for i in range(n_layers)]).astype(ml_dtypes.bfloat16)
    avec = np.stack([np.stack([att_src[i], att_dst[i]], axis=1)
                     for i in range(n_layers)]).astype(ml_dtypes.bfloat16)
    biasr = np.stack([np.tile(bias[i], (128, 1)) for i in range(n_layers)]).astype(np.float32)
    lwb = np.stack([lin_w[i * HID:(i + 1) * HID] for i in range(n_layers)]).astype(ml_dtypes.bfloat16)
    linb = lin_b.reshape(128, 1).astype(np.float32)
    identb = np.eye(128, dtype=ml_dtypes.bfloat16)
    identf = np.eye(128, dtype=np.float32)
    onesb = np.ones((128, 128), np.float32)
    padm = np.zeros((128, 1), np.float32)
    padm[:PAD_SLOT0] = 1.0
    padz = np.zeros((128, 1), np.float32)
    padz[PAD_SLOT0:] = -200.0

    # xag0: full x in padded transposed block form [8*128, NSHP]
    xag0 = np.zeros((NC * 128, NSHP), ml_dtypes.bfloat16)
    blocks = []
    for c in range(NC):
        xs = x[c * NSH:(c + 1) * NSH][percore[c]["order"]]
        blk = np.zeros((128, NSHP), np.float32)
        blk[:, :NSH] = xs.T
        blocks.append(blk)
        xag0[c * 128:(c + 1) * 128] = blk.astype(ml_dtypes.bfloat16)

    in_maps = []
    for c in range(NC):
        in_maps.append({
            "xag0": xag0, "ownx0": xag0[c * 128:(c + 1) * 128],
            "idx": percore[c]["idx"],
            "Wb": Wb, "WTb": WTb, "avec": avec, "biasr": biasr,
            "lwb": lwb, "linb": linb, "identb": identb, "identf": identf,
            "onesb": onesb, "padm": padm, "padz": padz,
        })
    return in_maps


def assemble_output(results, percore):
    out = np.empty((N, HID), np.float32)
    for c in range(NC):
        order = percore[c]["order"]
        yc = results[c]["y"][:NSH]
        out[c * NSH + order] = yc
    return out


_CACHE = {}


def _get_compiled(edge_key, edge_index, n_layers=L):
    key = (edge_key, n_layers)
    if key not in _CACHE:
        meta, percore = preprocess(edge_index)
        nc = bacc.Bacc("TRN2", target_bir_lowering=False, debug=False,
                       num_devices=NC)
        build(nc, meta, n_layers=n_layers)
        nc.compile()
        _CACHE[key] = (nc, meta, percore)
    return _CACHE[key]


def kernel(**inputs):
    from concourse.bass_utils import run_bass_kernel_spmd
    edge_index = np.asarray(inputs["edge_index"])
    edge_key = hash(edge_index.tobytes())
    nc, meta, percore = _get_compiled(edge_key, edge_index)
    in_maps = make_inputs(inputs, meta, percore, n_layers=L)
    res = run_bass_kernel_spmd(nc, in_maps, list(range(NC)))
    return assemble_output(res.results, percore)


# revision 4
# speedup vs baseline: 1.0142x; 1.0142x over previous
"""GAT encoder (10-layer, JK-concat) Trainium2 Bass kernel v2 — 8-core.

One collective per layer: AllGather PRE-pairnorm aggregated rows (transposed
form, bf16) with f32 stats packed via bitcast into extra columns. Each core
then runs pairnorm+gelu+dense redundantly over the full 50176-node table to
build its local gather table [h(128) | one | pad | zs(f32-as-2bf16)].
Attention logit zs is gathered WITH the features (no per-edge dots); the
softmax denominator rides as a ones-column through the PE accumulate matmul.
Pads point at dedicated zs=-200 rows so their exp weight underflows to 0.
"""

import numpy as np
import ml_dtypes
from contextlib import ExitStack

import concourse.bass as bass
import concourse.bacc as bacc
import concourse.tile as tile
import concourse.mybir as mybir

F32 = mybir.dt.float32
F32R = mybir.dt.float32r
BF16 = mybir.dt.bfloat16
I16 = mybir.dt.int16
AX = mybir.AxisListType
OP = mybir.AluOpType
AF = mybir.ActivationFunctionType

N = 50000
E = 640000
HID = 128
L = 10
NC = 8
NSH = N // NC          # 6250
TILES = 49
NSHP = TILES * 128     # 6272
TBL = NSHP * NC        # 50176
HI_BASE = TBL - 32768  # 17408
NEG = 0.2
PEPS = 1e-5
SEPS = 1e-16
SEG_MAX_ROUNDS = 48
ROWW = 256             # table row (bf16): h(128)|one|pad|zs f32|pad... 512B
GW = 256               # gather window = table row
PSW = 132              # psum row width: h(128) | 0 | 0 | zs | zd
OWNW = 134             # own row:   h(128) | one | pad | zs f32 | zd f32
ZS_F32 = 65            # f32 col of zs within a row (bytes 260..263)
PAD_LO_ROW = 6250          # core0 pad row (zs=-200), in lo window
PAD_HI_ROW = TBL - 22      # 50154 core7 pad row; 50154-17408=32746 <= 32767
PAD_SLOT0 = NSH - (TILES - 1) * 128   # 106: first pad slot in tile 48
STATS_COLS = 4
AGW = NSHP + STATS_COLS
# stream chunks per block (cols), multiples of 128
CHUNKS = [(0, 1664), (1664, 3328), (3328, 4992), (4992, 6272)]
import os
NO_COLL = os.environ.get("K2_NO_COLL", "0") == "1"
PAIR_MM = os.environ.get("K2_PAIR_MM", "0") == "1" 


def preprocess(edge_index):
    """Static graph preprocessing (no mask needed in v2)."""
    src = np.asarray(edge_index[0], dtype=np.int64)
    dst = np.asarray(edge_index[1], dtype=np.int64)
    owner = dst // NSH

    orders = []
    inv_all = np.empty(N, np.int64)
    for c in range(NC):
        m = owner == c
        dloc = dst[m] - c * NSH
        deg = np.bincount(dloc, minlength=NSH)
        order = np.argsort(-deg, kind="stable")
        inv = np.empty(NSH, np.int64)
        inv[order] = np.arange(NSH)
        orders.append(order)
        inv_all[c * NSH:(c + 1) * NSH] = inv
    tblrow_of_src = (src // NSH) * NSHP + inv_all[src]

    lo_lists = [[[] for _ in range(NSHP)] for _ in range(NC)]
    hi_lists = [[[] for _ in range(NSHP)] for _ in range(NC)]
    for c in range(NC):
        m = owner == c
        rows = tblrow_of_src[m]
        dpos = inv_all[dst[m]]
        o = np.argsort(dpos, kind="stable")
        rows = rows[o]
        dpos = dpos[o]
        counts = np.bincount(dpos, minlength=NSH)
        starts = np.concatenate([[0], np.cumsum(counts)])
        for p in range(NSH):
            r = rows[starts[p]:starts[p + 1]]
            ml = r[r < HI_BASE]
            mh = r[r > 32767]
            fx = r[(r >= HI_BASE) & (r <= 32767)]
            nl, nh = len(ml), len(mh)
            lo_e, hi_e = [], []
            for v in fx:
                if nl <= nh:
                    lo_e.append(v); nl += 1
                else:
                    hi_e.append(v); nh += 1
            lo_lists[c][p] = np.concatenate([ml, np.array(lo_e, np.int64)]) if (len(ml) + len(lo_e)) else np.empty(0, np.int64)
            hi_lists[c][p] = np.concatenate([mh, np.array(hi_e, np.int64)]) if (len(mh) + len(hi_e)) else np.empty(0, np.int64)

    D_lo = np.zeros(TILES, np.int64)
    D_hi = np.zeros(TILES, np.int64)
    for t in range(TILES):
        for c in range(NC):
            for sl in range(128):
                p = t * 128 + sl
                D_lo[t] = max(D_lo[t], len(lo_lists[c][p]))
                D_hi[t] = max(D_hi[t], len(hi_lists[c][p]))
    rounds_tot = int((D_lo + D_hi).sum())

    segs = []
    cur, cur_r = [], 0
    for t in range(TILES):
        rt = int(D_lo[t] + D_hi[t])
        if cur and cur_r + rt > SEG_MAX_ROUNDS:
            segs.append(cur)
            cur, cur_r = [], 0
        cur.append(t)
        cur_r += rt
    if cur:
        segs.append(cur)

    def wrap_idx(flat):
        n = len(flat)
        assert n % 16 == 0
        w = np.asarray(flat, np.int16).reshape(-1, 16).T
        return np.tile(w, (8, 1))

    percore = []
    for c in range(NC):
        idx_blocks = []
        for seg in segs:
            for part, D in (("lo", D_lo), ("hi", D_hi)):
                flat = []
                lists = lo_lists[c] if part == "lo" else hi_lists[c]
                base = 0 if part == "lo" else HI_BASE
                padv = PAD_LO_ROW if part == "lo" else PAD_HI_ROW - HI_BASE
                for t in seg:
                    for k in range(int(D[t])):
                        for sl in range(128):
                            p = t * 128 + sl
                            lst = lists[p]
                            if k < len(lst):
                                flat.append(int(lst[k]) - base)
                            else:
                                flat.append(padv)
                if flat:
                    idx_blocks.append(wrap_idx(flat))
        idx_all = np.concatenate(idx_blocks, axis=1) if idx_blocks else np.zeros((128, 1), np.int16)
        percore.append({"idx": idx_all, "order": orders[c]})

    meta = {"D_lo": D_lo, "D_hi": D_hi, "segs": segs, "rounds_tot": rounds_tot}
    return meta, percore


def build(nc, meta, n_layers=L):
    D_lo, D_hi, segs = meta["D_lo"], meta["D_hi"], meta["segs"]
    rounds_tot = meta["rounds_tot"]

    # ---- DRAM tensors
    xag0_in = nc.dram_tensor("xag0", [NC * 128, NSHP], BF16, kind="ExternalInput")
    ownx0_in = nc.dram_tensor("ownx0", [128, NSHP], BF16, kind="ExternalInput")
    idx_in = nc.dram_tensor("idx", [128, 8 * rounds_tot], I16, kind="ExternalInput")
    Wb_in = nc.dram_tensor("Wb", [n_layers, 128, 128], BF16, kind="ExternalInput")
    WTb_in = nc.dram_tensor("WTb", [n_layers, 128, 128], BF16, kind="ExternalInput")
    avec_in = nc.dram_tensor("avec", [n_layers, 128, 2], BF16, kind="ExternalInput")
    biasr_in = nc.dram_tensor("biasr", [n_layers, 128, 128], F32, kind="ExternalInput")
    lwb_in = nc.dram_tensor("lwb", [n_layers, 128, 128], BF16, kind="ExternalInput")
    linb_in = nc.dram_tensor("linb", [128, 1], F32, kind="ExternalInput")
    identb_in = nc.dram_tensor("identb", [128, 128], BF16, kind="ExternalInput")
    identf_in = nc.dram_tensor("identf", [128, 128], F32, kind="ExternalInput")
    onesb_in = nc.dram_tensor("onesb", [128, 128], F32, kind="ExternalInput")
    padm_in = nc.dram_tensor("padm", [128, 1], F32, kind="ExternalInput")
    padz_in = nc.dram_tensor("padz", [128, 1], F32, kind="ExternalInput")
    y_out = nc.dram_tensor("y", [NSHP, 128], F32, kind="ExternalOutput")

    ag_in = nc.dram_tensor("ag_in", [128, AGW], BF16)
    xag = nc.dram_tensor("xag", [NC * 128, AGW], BF16, addr_space="Shared")
    ag_st = nc.dram_tensor("ag_st", [128, STATS_COLS], BF16)
    xag_st = nc.dram_tensor("xag_st", [NC * 128, STATS_COLS], BF16,
                            addr_space="Shared")
    table = nc.dram_tensor("table", [TBL, ROWW], BF16)

    RG = [list(range(NC))]

    with tile.TileContext(nc) as tc, ExitStack() as ctx:
        P = ctx.enter_context(tc.tile_pool(name="persist", bufs=1))
        idx_sb = P.tile([128, 8 * rounds_tot], I16, tag="idx")
        Wb = P.tile([128, n_layers * 128], BF16, tag="Wb")
        WTb = P.tile([128, n_layers * 128], BF16, tag="WTb")
        avec = P.tile([128, n_layers * 2], BF16, tag="avec")
        biasr = P.tile([128, n_layers * 128], F32, tag="biasr")
        lwb = P.tile([128, n_layers * 128], BF16, tag="lwb")
        linb = P.tile([128, 1], F32, tag="linb")
        identb = P.tile([128, 128], BF16, tag="identb")
        identf = P.tile([128, 128], F32, tag="identf")
        onesb = P.tile([128, 128], F32, tag="onesb")
        padm = P.tile([128, 1], F32, tag="padm")
        padz = P.tile([128, 1], F32, tag="padz")
        ownxT = P.tile([128, NSHP], BF16, tag="ownxT")
        ownh = P.tile([128, TILES * OWNW], BF16, tag="ownh")
        payload = P.tile([128, AGW], BF16, tag="payload")
        outfinT = P.tile([128, NSHP], F32, tag="outfinT")
        wself = P.tile([128, TILES], F32, tag="wself")
        sqparts = P.tile([128, 13], F32, tag="sqparts")
        invdr = P.tile([128, 1], F32, tag="invdr")
        nms = P.tile([128, 1], F32, tag="nms")
        Waug = P.tile([128, PSW], BF16, tag="Waug")
        scr = P.tile([128, 512], F32, tag="scr")

        nc.sync.dma_start(idx_sb[:], idx_in.ap())
        nc.sync.dma_start(Wb[:].rearrange("a (l b) -> a l b", b=128), Wb_in.ap().rearrange("l a b -> a l b"))
        nc.sync.dma_start(WTb[:].rearrange("a (l b) -> a l b", b=128), WTb_in.ap().rearrange("l a b -> a l b"))
        nc.sync.dma_start(avec[:].rearrange("a (l b) -> a l b", b=2), avec_in.ap().rearrange("l a b -> a l b"))
        nc.sync.dma_start(biasr[:].rearrange("a (l b) -> a l b", b=128), biasr_in.ap().rearrange("l a b -> a l b"))
        nc.sync.dma_start(lwb[:].rearrange("a (l b) -> a l b", b=128), lwb_in.ap().rearrange("l a b -> a l b"))
        nc.sync.dma_start(linb[:], linb_in.ap())
        nc.sync.dma_start(identb[:], identb_in.ap())
        nc.sync.dma_start(identf[:], identf_in.ap())
        nc.sync.dma_start(onesb[:], onesb_in.ap())
        nc.sync.dma_start(padm[:], padm_in.ap())
        nc.sync.dma_start(padz[:], padz_in.ap())
        nc.sync.dma_start(ownxT[:], ownx0_in.ap())

        PR = ctx.enter_context(tc.tile_pool(name="pr", bufs=2, space="PSUM"))
        PA = ctx.enter_context(tc.tile_pool(name="pa", bufs=2, space="PSUM"))
        PT = ctx.enter_context(tc.tile_pool(name="pt", bufs=2, space="PSUM"))
        GSEG = ctx.enter_context(tc.tile_pool(name="gseg", bufs=2))
        XIN = ctx.enter_context(tc.tile_pool(name="xin", bufs=2))
        RS = ctx.enter_context(tc.tile_pool(name="rs", bufs=2))
        SC = ctx.enter_context(tc.tile_pool(name="sc", bufs=4))
        ST = ctx.enter_context(tc.tile_pool(name="st", bufs=6))
        RP = ctx.enter_context(tc.tile_pool(name="rp", bufs=4))

        # idx col offsets per segment (wrapped layout: 8 cols per round)
        idx_off = {}
        ic = 0
        for si, seg in enumerate(segs):
            lo_r = int(sum(D_lo[t] for t in seg))
            hi_r = int(sum(D_hi[t] for t in seg))
            idx_off[si] = (ic, lo_r, ic + 8 * lo_r, hi_r)
            ic += 8 * (lo_r + hi_r)

        def stats_prologue(from_st=False):
            gst = ST.tile([128, 4 * NC], BF16, tag="gst")
            src_ap = (xag_st.ap() if from_st
                      else xag.ap()[:, NSHP:NSHP + 4])
            nc.sync.dma_start(
                gst[:].rearrange("p (c f) -> p c f", f=4),
                src_ap.rearrange("(c p) f -> p c f", p=128))
            gf = gst[:].bitcast(F32).rearrange("p (c f) -> p c f", f=2)
            Sx = ST.tile([128, 1], F32, tag="Sx")
            nc.vector.reduce_sum(out=Sx[:], in_=gf[:, :, 0:1].squeeze(2), axis=AX.X)
            Sq = ST.tile([128, 1], F32, tag="Sq")
            nc.vector.reduce_sum(out=Sq[:], in_=gf[:, :, 1:2].squeeze(2), axis=AX.X)
            mu = ST.tile([128, 1], F32, tag="mu")
            nc.vector.tensor_scalar_mul(mu[:], Sx[:], 1.0 / N)
            p1 = PT.tile([128, 128], F32, tag="pt")
            nc.tensor.matmul(p1[:1, :1], Sq[:], onesb[:, :1], start=True, stop=True)
            p2 = PT.tile([128, 128], F32, tag="pt")
            nc.tensor.matmul(p2[:1, :1], mu[:], mu[:], start=True, stop=True)
            sqt = ST.tile([1, 2], F32, tag="sqt")
            nc.vector.tensor_copy(sqt[:, 0:1], p1[:1, :1])
            nc.vector.tensor_copy(sqt[:, 1:2], p2[:1, :1])
            v3 = ST.tile([1, 1], F32, tag="v3")
            nc.vector.tensor_scalar(v3[:], sqt[:, 0:1], 1.0 / N, PEPS, op0=OP.mult, op1=OP.add)
            v4 = ST.tile([1, 1], F32, tag="v4")
            nc.vector.tensor_tensor(v4[:], v3[:], sqt[:, 1:2], op=OP.subtract)
            den = ST.tile([1, 1], F32, tag="den")
            nc.scalar.activation(den[:], v4[:], AF.Sqrt)
            invd = ST.tile([1, 1], F32, tag="invd")
            nc.vector.reciprocal(invd[:], den[:])
            pb = PT.tile([128, 128], F32, tag="pt")
            nc.tensor.matmul(pb[:, :1], onesb[:1, :], invd[:], start=True, stop=True)
            nc.vector.tensor_copy(invdr[:], pb[:, :1])
            nc.vector.tensor_scalar(nms[:], mu[:], invdr[:], -1.0, op0=OP.mult, op1=OP.mult)

        def build_waug(l):
            pw = PT.tile([128, 128], F32, tag="pt")
            nc.tensor.matmul(pw[:, :2], WTb[:, l * 128:(l + 1) * 128],
                             avec[:, l * 2:(l + 1) * 2], start=True, stop=True)
            nc.vector.tensor_copy(Waug[:, :128], Wb[:, l * 128:(l + 1) * 128])
            nc.vector.memset(Waug[:, 128:130], 0)
            nc.vector.tensor_copy(Waug[:, 130:132], pw[:, :2])

        def stream_block(l, c):
            """Build table rows for block c of the full node table."""
            src_hbm = xag0_in if l == 0 else xag
            for (c0, c1) in CHUNKS:
                ncols = c1 - c0
                ntl = ncols // 128
                xin = XIN.tile([128, 1664], BF16, tag="xin")
                nc.sync.dma_start(xin[:, :ncols],
                                  src_hbm.ap()[c * 128:(c + 1) * 128, c0:c1])
                if l == 0:
                    xs = xin
                else:
                    xs = XIN.tile([128, 1664], BF16, tag="xs")
                    nc.scalar.activation(xs[:, :ncols], xin[:, :ncols],
                                         AF.Gelu, bias=nms[:], scale=invdr[:])
                rows = RS.tile([128, 13 * ROWW], BF16, tag="rows")
                for g0 in range(0, ntl, 3):
                    gn = min(3, ntl - g0)
                    pr = PR.tile([128, 3 * PSW], F32, tag="pr")
                    for j in range(gn):
                        nc.tensor.matmul(
                            pr[:, j * PSW:j * PSW + PSW],
                            xs[:, (g0 + j) * 128:(g0 + j + 1) * 128],
                            Waug[:], start=True, stop=True)
                    pr3 = pr[:].rearrange("p (k f) -> p k f", f=PSW)
                    r3 = rows[:, g0 * ROWW:].rearrange("p (k f) -> p k f", f=ROWW)
                    if g0 % 2 == 0:
                        nc.vector.tensor_copy(r3[:, :gn, :128], pr3[:, :gn, :128])
                    else:
                        nc.scalar.activation(r3[:, :gn, :128], pr3[:, :gn, :128], AF.Copy)
                    nc.vector.memset(r3[:, :gn, 128:130], 1.0)
                    rzf = rows[:].bitcast(F32).rearrange("p (k f) -> p k f", f=ROWW // 2)
                    nc.scalar.activation(rzf[:, g0:g0 + gn, 65:66],
                                         pr3[:, :gn, 130:131], AF.Copy)
                # pad-row zs fix on global tile 48 of this block (last chunk)
                if c1 == NSHP:
                    jt = ntl - 1
                    zv = rows[:].bitcast(F32).rearrange(
                        "p (k f) -> p k f", f=ROWW // 2)[:, jt:jt + 1, 65:66].squeeze(2)
                    nc.vector.tensor_scalar(zv, zv, padm[:], padz[:],
                                            op0=OP.mult, op1=OP.add)
                nc.sync.dma_start(
                    table.ap()[c * NSHP + c0:c * NSHP + c1, :].rearrange(
                        "(k p) f -> p k f", p=128),
                    rows[:, :ntl * ROWW].rearrange("p (k f) -> p k f", f=ROWW))

        def own_dense(l):
            """ownh slab: h(128)|..|zs,zd(f32) per own tile from ownxT."""
            for g0 in range(0, TILES, 3):
                gn = min(3, TILES - g0)
                pr = PR.tile([128, 3 * PSW], F32, tag="pr")
                for j in range(gn):
                    nc.tensor.matmul(
                        pr[:, j * PSW:j * PSW + PSW],
                        ownxT[:, (g0 + j) * 128:(g0 + j + 1) * 128],
                        Waug[:], start=True, stop=True)
                pr3 = pr[:].rearrange("p (k f) -> p k f", f=PSW)
                o3 = ownh[:, g0 * OWNW:].rearrange("p (k f) -> p k f", f=OWNW)
                nc.vector.tensor_copy(o3[:, :gn, :128], pr3[:, :gn, :128])
                zof = ownh[:].bitcast(F32).rearrange("p (k f) -> p k f", f=OWNW // 2)
                nc.vector.tensor_copy(zof[:, g0:g0 + gn, 65:67],
                                      pr3[:, :gn, 130:132])

        def own_norm(l):
            # ownxT <- gelu(pairnorm(payload prenorm))  [l>=1]
            for c0 in range(0, NSHP, 512):
                cn = min(512, NSHP - c0)
                nc.scalar.activation(ownxT[:, c0:c0 + cn], payload[:, c0:c0 + cn],
                                     AF.Gelu, bias=nms[:], scale=invdr[:])

        def jk_increment(jl):
            for c0 in range(0, NSHP, 512):
                cn = min(512, NSHP - c0)
                pj = PA.tile([128, 512], F32, tag="pj")
                nc.tensor.matmul(pj[:, :cn], lwb[:, jl * 128:(jl + 1) * 128],
                                 ownxT[:, c0:c0 + cn], start=True, stop=True)
                if jl == 0:
                    nc.vector.tensor_copy(outfinT[:, c0:c0 + cn], pj[:, :cn])
                else:
                    nc.vector.tensor_tensor(outfinT[:, c0:c0 + cn],
                                            outfinT[:, c0:c0 + cn], pj[:, :cn], op=OP.add)

        def wself_compute():
            ozf = ownh[:].bitcast(F32).rearrange("p (k f) -> p k f", f=OWNW // 2)
            zsum = ST.tile([128, TILES], F32, tag="zsum")
            nc.vector.tensor_tensor(zsum[:], ozf[:, :, 65:66].squeeze(2),
                                    ozf[:, :, 66:67].squeeze(2), op=OP.add)
            zl = ST.tile([128, TILES], F32, tag="zlw")
            nc.vector.scalar_tensor_tensor(out=zl[:], in0=zsum[:], scalar=NEG,
                                           in1=zsum[:], op0=OP.mult, op1=OP.max)
            nc.scalar.activation(wself[:], zl[:], AF.Exp)

        def aggregate(l, last=False):
            for si, seg in enumerate(segs):
                lo_r = int(sum(D_lo[t] for t in seg))
                hi_r = int(sum(D_hi[t] for t in seg))
                seg_r = lo_r + hi_r
                gbuf = GSEG.tile([128, SEG_MAX_ROUNDS * GW], BF16, tag="gseg")
                g3 = gbuf[:].rearrange("p (r e) -> p r e", e=GW)
                ic_lo, nlo, ic_hi, nhi = idx_off[si]
                if lo_r:
                    nc.gpsimd.dma_gather(
                        g3[:, :lo_r, :], table.ap()[:32768, :],
                        idx_sb[:, ic_lo:ic_lo + 8 * lo_r],
                        128 * lo_r, 128 * lo_r, GW, single_packet=False)
                if hi_r:
                    nc.gpsimd.dma_gather(
                        g3[:, lo_r:seg_r, :], table.ap()[HI_BASE:TBL, :],
                        idx_sb[:, ic_hi:ic_hi + 8 * hi_r],
                        128 * hi_r, 128 * hi_r, GW, single_packet=False)

                ozf = ownh[:].bitcast(F32).rearrange("p (k f) -> p k f", f=OWNW // 2)
                zdexp = SC.tile([128, SEG_MAX_ROUNDS], F32, tag="zdexp")
                lo_cum, hi_cum = 0, 0
                tile_ranges = []
                for t in seg:
                    dlo, dhi = int(D_lo[t]), int(D_hi[t])
                    zd = ozf[:, t:t + 1, 66:67].squeeze(2)
                    if dlo:
                        nc.vector.tensor_copy(
                            zdexp[:, lo_cum:lo_cum + dlo],
                            zd.broadcast_to([128, dlo]))
                    if dhi:
                        nc.vector.tensor_copy(
                            zdexp[:, lo_r + hi_cum:lo_r + hi_cum + dhi],
                            zd.broadcast_to([128, dhi]))
                    tile_ranges.append((t, lo_cum, dlo, lo_r + hi_cum, dhi))
                    lo_cum += dlo
                    hi_cum += dhi

                zraw = gbuf[:].bitcast(F32).rearrange(
                    "p (r e) -> p r e", e=GW // 2)[:, :seg_r, 65:66].squeeze(2)
                zt = SC.tile([128, SEG_MAX_ROUNDS], F32, tag="zt")
                nc.vector.tensor_tensor(zt[:, :seg_r], zraw, zdexp[:, :seg_r], op=OP.add)
                zl = SC.tile([128, SEG_MAX_ROUNDS], F32, tag="zl")
                nc.vector.scalar_tensor_tensor(out=zl[:, :seg_r], in0=zt[:, :seg_r],
                                               scalar=NEG, in1=zt[:, :seg_r],
                                               op0=OP.mult, op1=OP.max)
                ew = SC.tile([128, SEG_MAX_ROUNDS], F32, tag="ew")
                nc.scalar.activation(ew[:, :seg_r], zl[:, :seg_r], AF.Exp)
                # apply weights in place on h+one cols
                gho = g3[:, :seg_r, 0:129]
                nc.vector.tensor_tensor(
                    gho, gho,
                    ew[:, :seg_r].unsqueeze(2).broadcast_to([128, seg_r, 129]),
                    op=OP.mult)

                for (t, lc, dlo, hc, dhi) in tile_ranges:
                    Dt = dlo + dhi
                    if PAIR_MM and dlo >= 2 and dhi >= 2:
                        pa = PA.tile([128, 258], F32, tag="pa2")
                        pa3 = pa[:].rearrange("p (a b) -> p a b", b=129)
                        mms = []
                        for (base, d) in ((lc, dlo), (hc, dhi)):
                            for k in range(0, d - 1, 2):
                                mms.append((g3[:, base + k:base + k + 2, 0:129], True))
                            if d % 2:
                                mms.append((gbuf[:, (base + d - 1) * GW:
                                                 (base + d - 1) * GW + 129], False))
                        for i, (mv, dual) in enumerate(mms):
                            nc.tensor.matmul(pa3[:, :, :] if dual else pa[:, :129],
                                             identb[:], mv,
                                             start=(i == 0), stop=(i == len(mms) - 1),
                                             skip_group_check=True)
                        mg = RP.tile([128, 129], F32, tag="mg")
                        nc.vector.tensor_tensor(mg[:], pa[:, :129], pa[:, 129:258],
                                                op=OP.add)
                        pa_h, pa_s = mg[:, :128], mg[:, 128:129]
                    else:
                        pa = PA.tile([128, 129], F32, tag="pa")
                        ks = [lc + k for k in range(dlo)] + [hc + k for k in range(dhi)]
                        for i, kc in enumerate(ks):
                            nc.tensor.matmul(pa[:], identb[:],
                                             gbuf[:, kc * GW:kc * GW + 129],
                                             start=(i == 0), stop=(i == Dt - 1))
                        pa_h, pa_s = pa[:, :128], pa[:, 128:129]
                    Sp = ST.tile([128, 1], F32, tag="Sp")
                    nc.vector.tensor_scalar(Sp[:], pa_s, wself[:, t:t + 1],
                                            SEPS, op0=OP.add, op1=OP.add)
                    rec = ST.tile([128, 1], F32, tag="rec")
                    nc.vector.reciprocal(rec[:], Sp[:])
                    acc1 = RP.tile([128, 128], F32, tag="acc1")
                    nc.vector.scalar_tensor_tensor(
                        out=acc1[:], in0=ownh[:, t * OWNW:t * OWNW + 128],
                        scalar=wself[:, t:t + 1], in1=pa_h,
                        op0=OP.mult, op1=OP.add)
                    row = RP.tile([128, 128], F32, tag="row")
                    nc.vector.scalar_tensor_tensor(
                        out=row[:], in0=acc1[:], scalar=rec[:],
                        in1=biasr[:, l * 128:(l + 1) * 128], op0=OP.mult, op1=OP.add)
                    if t == TILES - 1:
                        nc.vector.tensor_scalar_mul(row[:], row[:], padm[:])
                    ptf = PT.tile([128, 128], F32, tag="pt")
                    nc.tensor.transpose(ptf[:], row[:], identf[:])
                    nc.scalar.activation(payload[:, t * 128:(t + 1) * 128], ptf[:],
                                         AF.Copy)
            # stats from payload (per-feature sums; per-partition sumsq partials)
            pzf = payload[:].bitcast(F32)
            fsum = ST.tile([128, 1], F32, tag="fsum")
            nc.vector.reduce_sum(out=fsum[:], in_=payload[:, :NSHP], axis=AX.X)
            nc.vector.tensor_copy(pzf[:, NSHP // 2:NSHP // 2 + 1], fsum[:])
            for i, c0 in enumerate(range(0, NSHP, 512)):
                cn = min(512, NSHP - c0)
                nc.scalar.activation(scr[:, :cn], payload[:, c0:c0 + cn],
                                     AF.Square, accum_out=sqparts[:, i:i + 1])
            spt = ST.tile([128, 1], F32, tag="spt")
            nc.vector.reduce_sum(out=spt[:], in_=sqparts[:], axis=AX.X)
            nc.vector.tensor_copy(pzf[:, NSHP // 2 + 1:NSHP // 2 + 2], spt[:])
            if last:
                nc.sync.dma_start(ag_st.ap(), payload[:, NSHP:NSHP + STATS_COLS])
                nc.gpsimd.collective_compute(
                    "AllGather", OP.bypass, replica_groups=RG,
                    ins=[ag_st.ap()], outs=[xag_st.ap()])
                return
            nc.sync.dma_start(ag_in.ap(), payload[:])
            if NO_COLL:
                for c in range(NC):
                    nc.sync.dma_start(xag.ap()[c * 128:(c + 1) * 128, :], ag_in.ap())
            else:
                nc.gpsimd.collective_compute(
                    "AllGather", OP.bypass, replica_groups=RG,
                    ins=[ag_in.ap()], outs=[xag.ap()])

        # ================= main layer loop =================
        for l in range(n_layers):
            if l > 0:
                stats_prologue()
                own_norm(l)
            build_waug(l)
            # own dense first (aggregation needs ownh/wself)
            own_dense(l)
            wself_compute()
            if l > 0:
                jk_increment(l - 1)
            for c in range(NC):
                stream_block(l, c)
            aggregate(l, last=(l == n_layers - 1))

        # ================= epilogue pass =================
        stats_prologue(from_st=True)
        own_norm(n_layers)
        jk_increment(n_layers - 1)
        nc.vector.tensor_scalar_add(outfinT[:], outfinT[:], linb[:])
        for t0 in range(0, TILES, 13):
            tn = min(13, TILES - t0)
            yrows = RS.tile([128, 13 * 128], F32, tag="yrows")
            for j in range(tn):
                pt = PT.tile([128, 128], F32, tag="pt")
                nc.tensor.transpose(pt[:], outfinT[:, (t0 + j) * 128:(t0 + j + 1) * 128], identf[:])
                nc.vector.tensor_copy(yrows[:, j * 128:(j + 1) * 128], pt[:])
            nc.sync.dma_start(
                y_out.ap()[t0 * 128:(t0 + tn) * 128, :].rearrange("(t p) f -> p t f", p=128),
                yrows[:, :tn * 128].rearrange("p (t f) -> p t f", f=128))

    return nc


def make_inputs(inputs, meta, percore, n_layers=L):
    x = np.asarray(inputs["x"], np.float32)
    W0 = np.asarray(inputs["W0"], np.float32)
    Ws = np.asarray(inputs["Ws"], np.float32)
    att_src = np.asarray(inputs["att_src"], np.float32)
    att_dst = np.asarray(inputs["att_dst"], np.float32)
    bias = np.asarray(inputs["bias"], np.float32)
    lin_w = np.asarray(inputs["lin_w"], np.float32)
    lin_b = np.asarray(inputs["lin_b"], np.float32)

    Wst = np.stack([W0] + [Ws[i] for i in range(n_layers - 1)])
    Wb = Wst.astype(ml_dtypes.bfloat16)
    WTb = np.stack([Wst[i].T for i in range(n_layers)]).astype(ml_dtypes.bfloat16)
    avec = np.stack([np.stack([att_src[i], att_dst[i]], axis=1)
                     for i in range(n_layers)]).astype(ml_dtypes.bfloat16)
    biasr = np.stack([np.tile(bias[i], (128, 1)) for i in range(n_layers)]).astype(np.float32)
    lwb = np.stack([lin_w[i * HID:(i + 1) * HID] for i in range(n_layers)]).astype(ml_dtypes.bfloat16)
    linb = lin_b.reshape(128, 1).astype(np.float32)
    identb = np.eye(128, dtype=ml_dtypes.bfloat16)
    identf = np.eye(128, dtype=np.float32)
    onesb = np.ones((128, 128), np.float32)
    padm = np.zeros((128, 1), np.float32)
    padm[:PAD_SLOT0] = 1.0
    padz = np.zeros((128, 1), np.float32)
    padz[PAD_SLOT0:] = -200.0

    # xag0: full x in padded transposed block form [8*128, NSHP]
    xag0 = np.zeros((NC * 128, NSHP), ml_dtypes.bfloat16)
    blocks = []
    for c in range(NC):
        xs = x[c * NSH:(c + 1) * NSH][percore[c]["order"]]
        blk = np.zeros((128, NSHP), np.float32)
        blk[:, :NSH] = xs.T
        blocks.append(blk)
        xag0[c * 128:(c + 1) * 128] = blk.astype(ml_dtypes.bfloat16)

    in_maps = []
    for c in range(NC):
        in_maps.append({
            "xag0": xag0, "ownx0": xag0[c * 128:(c + 1) * 128],
            "idx": percore[c]["idx"],
            "Wb": Wb, "WTb": WTb, "avec": avec, "biasr": biasr,
            "lwb": lwb, "linb": linb, "identb": identb, "identf": identf,
            "onesb": onesb, "padm": padm, "padz": padz,
        })
    return in_maps


def assemble_output(results, percore):
    out = np.empty((N, HID), np.float32)
    for c in range(NC):
        order = percore[c]["order"]
        yc = results[c]["y"][:NSH]
        out[c * NSH + order] = yc
    return out


_CACHE = {}


def _get_compiled(edge_key, edge_index, n_layers=L):
    key = (edge_key, n_layers)
    if key not in _CACHE:
        meta, percore = preprocess(edge_index)
        nc = bacc.Bacc("TRN2", target_bir_lowering=False, debug=False,
                       num_devices=NC)
        build(nc, meta, n_layers=n_layers)
        nc.compile()
        _CACHE[key] = (nc, meta, percore)
    return _CACHE[key]


def kernel(**inputs):
    from concourse.bass_utils import run_bass_kernel_spmd
    edge_index = np.asarray(inputs["edge_index"])
    edge_key = hash(edge_index.tobytes())
    nc, meta, percore = _get_compiled(edge_key, edge_index)
    in_maps = make_inputs(inputs, meta, percore, n_layers=L)
    res = run_bass_kernel_spmd(nc, in_maps, list(range(NC)))
    return assemble_output(res.results, percore)
